# revision 4
# baseline (speedup 1.0000x reference)
"""Multi-head attention TRN2 Bass kernel, batch-parallel over 8 NeuronCores.

Problem (hardcoded): B=8, Sq=Sk=1024, D_MODEL=1024, H=16, DK=DV=64,
DIM_OUT=1024, f32.  mask (B,1,1,Sk) marks keys to suppress (x*-1e9).
Returns (out, attn) like the reference.

Per-core plan (core b handles batch element b; no collectives):
  A) load+PE-transpose v,q,k (streamed in seq halves)
  B) projections:
       qhT = (q@wq+bq)^T   [H*DK, Sq]   (transposed layout)
       khT = (k@wk+bk)^T   [H*DK, Sk]
       vh  = v@wv+bv       [Sk, H*(DV+1)] (natural; per-head 65th col = 1.0
                                           so PV also row-sums exp -> L)
  C) per head pair (2 heads share a 128-partition chunk; K=64 matmuls of the
     two heads pack onto PE row groups):
     C1 transposed pass: S^T[j,i] tiles -> exp(0.125*S^T + NEG*mask[j]) (mask
        as per-partition ACT bias) -> P^T -> PV: ctx^T[d,i] (+L row via the
        ones column).  ctx normalized by 1/L (k=1 ones-matmul broadcast)
        into ctxT (float32r) for the output projection.
     C2 normal pass: S[i,j] tiles (+ mask*NEG*8 added via k=1 matmul
        accumulate, pre-scale) -> exp(0.125*S') -> *1/L[i] (per-partition
        scalar) -> DMA to attn output.
  D) out = ctxN^T.T @ wo + bo -> DMA out.

All matmuls run in float32r (fp32 stored, PE rounds mantissa, 1 cycle/row);
PSUM accumulation is fp32.
"""

import numpy as np
from contextlib import ExitStack

import concourse.bass as bass
import concourse.tile as tile
from concourse import bacc, mybir
from concourse import bass_utils

F32 = mybir.dt.float32
F32R = mybir.dt.float32r
EXP = mybir.ActivationFunctionType.Exp

P = 128          # partitions
S = 1024         # seq len (q and k)
D = 1024         # d_model
H = 16           # heads
DK = 64          # head dim
NB = 512         # matmul moving-dim block (fp32 max)
SC = S // P      # 8 seq chunks
DC = D // P      # 8 d_model chunks
NEG = -1e9
SCALE = 0.125    # 1/sqrt(DK)
N_CORES = 8

# bias rows inside the consolidated [P, S] bias tiles
BROW = {"bq": 0, "bk": 32, "bv": 64, "bo": 96}

_CACHE = {}


def _tile_pos(r):
    return (r, 0) if r == 96 else None


def _build_module():
    nc = bacc.Bacc("TRN2", target_bir_lowering=False, debug=False)

    q_d = nc.dram_tensor("q", [S, D], F32, kind="ExternalInput").ap()
    k_d = nc.dram_tensor("k", [S, D], F32, kind="ExternalInput").ap()
    v_d = nc.dram_tensor("v", [S, D], F32, kind="ExternalInput").ap()
    mask_d = nc.dram_tensor("mask", [S], F32, kind="ExternalInput").ap()
    wq_d = nc.dram_tensor("wq", [D, H * DK], F32, kind="ExternalInput").ap()
    wk_d = nc.dram_tensor("wk", [D, H * DK], F32, kind="ExternalInput").ap()
    wv_d = nc.dram_tensor("wv", [D, H * DK], F32, kind="ExternalInput").ap()
    wo_d = nc.dram_tensor("wo", [H * DK, D], F32, kind="ExternalInput").ap()
    b_ds = {
        "bq": nc.dram_tensor("bq", [H * DK], F32, kind="ExternalInput").ap(),
        "bk": nc.dram_tensor("bk", [H * DK], F32, kind="ExternalInput").ap(),
        "bv": nc.dram_tensor("bv", [H * DK], F32, kind="ExternalInput").ap(),
        "bo": nc.dram_tensor("bo", [D], F32, kind="ExternalInput").ap(),
    }
    ident_d = nc.dram_tensor("ident", [P, P], F32, kind="ExternalInput").ap()

    attn_d = nc.dram_tensor("attn", [H, S, S], F32, kind="ExternalOutput").ap()
    out_d = nc.dram_tensor("out", [S, D], F32, kind="ExternalOutput").ap()

    with tile.TileContext(nc) as tc, ExitStack() as top:
        const = top.enter_context(tc.tile_pool(name="const", bufs=1))
        persist = top.enter_context(tc.tile_pool(name="persist", bufs=1))

        # ---- constants ----
        ident_r = const.tile([P, P], F32R, tag="ident_r")
        nc.sync.dma_start(ident_r[:], ident_d.bitcast(F32R)[:])

        ones_f = const.tile([1, NB], F32, tag="ones_f")
        nc.vector.memset(ones_f[:], 1.0)
        ones4 = const.tile([P, NB], F32R, tag="ones4")
        for r in (0, 32, 64, 96):
            nc.vector.tensor_copy(ones4[r:r + 1, :], ones_f[:])
        ones16 = const.tile([P, H], F32, tag="ones16")
        nc.vector.memset(ones16[:], 1.0)

        mask_row = const.tile([1, S], F32, tag="mask_row")
        nc.sync.dma_start(mask_row[:], mask_d.rearrange("(a n) -> a n", a=1))
        mask4 = const.tile([P, S], F32R, tag="mask4")
        nc.vector.tensor_scalar_mul(mask4[0:1, :], mask_row[:], NEG * 8.0)
        for r in (32, 64, 96):
            nc.vector.tensor_copy(mask4[r:r + 1, :], mask4[0:1, :].bitcast(F32))

        # mask in column layout: [p, c] = mask[c*128+p] * NEG  (ACT bias, C1)
        mask_colneg = const.tile([P, SC], F32, tag="mask_colneg")
        nc.sync.dma_start(mask_colneg[:], mask_d.rearrange("(c p) -> p c", p=P))
        nc.vector.tensor_scalar_mul(mask_colneg[:], mask_colneg[:], NEG)

        # biases on partitions {0,32,64,96} of one [P, S] tile
        b4f = const.tile([P, S], F32, tag="b4f")
        b4r = const.tile([P, S], F32R, tag="b4r")
        for nm, r in BROW.items():
            nc.sync.dma_start(
                b4f[r:r + 1, :], b_ds[nm].rearrange("(a n) -> a n", a=1))
            nc.vector.tensor_copy(b4r[r:r + 1, :], b4f[r:r + 1, :])

        # ---- persistent big tensors ----
        qhT = [persist.tile([P, S], F32R, tag=f"qhT{c}", name=f"qhT{c}")
               for c in range(DC)]
        khT = [persist.tile([P, S], F32R, tag=f"khT{c}", name=f"khT{c}")
               for c in range(DC)]
        vh = [persist.tile([P, H * 65], F32R, tag=f"vh{c}", name=f"vh{c}")
              for c in range(SC)]
        ctxT = [persist.tile([P, S], F32R, tag=f"ctxT{c}", name=f"ctxT{c}")
                for c in range(DC)]

        # ================= A+B: transpose inputs and project =================
        with tc.tile_pool(name="ab_sbuf", bufs=1) as abp, \
             tc.tile_pool(name="ab_psum", bufs=1, space="PSUM") as abps:

            def do_tensor(x_d, w_d, bnm, mode, dstT):
                ws = []
                for c in range(DC):
                    wt = abp.tile([P, D], F32R, tag=f"w{c}", bufs=1,
                                  name=f"w{c}")
                    nc.sync.dma_start(
                        wt[:], w_d.bitcast(F32R)[c * P:(c + 1) * P, :])
                    ws.append(wt)
                br = BROW[bnm]
                for half in range(2):
                    # transpose seq rows [half*512, half*512+512) -> xT halves
                    xT = [abp.tile([P, NB], F32R, tag=f"xT{c}", bufs=1,
                                   name=f"xT{c}") for c in range(DC)]
                    for tp_i in range(2):
                        t0 = half * 4 + 2 * tp_i
                        stg0 = abp.tile([P, D], F32R, tag="stg0", bufs=1)
                        nc.sync.dma_start(
                            stg0[:], x_d.bitcast(F32R)[t0 * P:(t0 + 1) * P, :])
                        stg1 = abp.tile([P, D], F32R, tag="stg1", bufs=1)
                        nc.sync.dma_start(
                            stg1[:],
                            x_d.bitcast(F32R)[(t0 + 1) * P:(t0 + 2) * P, :])
                        for c in range(DC):
                            tp = abps.tile([P, 2 * P], F32R, tag="tp", bufs=4)
                            nc.tensor.transpose(
                                tp[:, 0:P], stg0[:, c * P:(c + 1) * P],
                                ident_r[:])
                            nc.tensor.transpose(
                                tp[:, P:2 * P], stg1[:, c * P:(c + 1) * P],
                                ident_r[:])
                            nc.vector.tensor_copy(
                                xT[c][:, tp_i * 2 * P:(tp_i + 1) * 2 * P],
                                tp[:])
                    if mode == "T":
                        # dstT[m][:, half*NB:...] = (x@w+b)^T block
                        nb = half
                        for m in range(DC):
                            ps = abps.tile([P, NB], F32, tag="proj", bufs=4)
                            for c in range(DC):
                                nc.tensor.matmul(
                                    ps[:], ws[c][:, m * P:(m + 1) * P],
                                    xT[c][:], start=(c == 0), stop=False)
                            nc.tensor.matmul(
                                ps[:], b4r[br:br + 1, m * P:(m + 1) * P],
                                ones4[br:br + 1, 0:NB], start=False, stop=True,
                                tile_position=_tile_pos(br))
                            nc.any.tensor_copy(
                                dstT[m][:, nb * NB:(nb + 1) * NB], ps[:])
                    else:
                        # natural layout: vh[j-tile m in this half][cols]
                        for m in range(half * 4, half * 4 + 4):
                            for nb in range(2):
                                ps = abps.tile([P, NB], F32, tag="proj",
                                               bufs=4)
                                for c in range(DC):
                                    nc.tensor.matmul(
                                        ps[:],
                                        xT[c][:, (m % 4) * P:(m % 4 + 1) * P],
                                        ws[c][:, nb * NB:(nb + 1) * NB],
                                        start=(c == 0), stop=False)
                                nc.tensor.matmul(
                                    ps[:], ones4[br:br + 1, 0:P],
                                    b4r[br:br + 1, nb * NB:(nb + 1) * NB],
                                    start=False, stop=True,
                                    tile_position=_tile_pos(br))
                                src = ps[:].rearrange("p (h c) -> p h c", c=DK)
                                dst = vh[m][:, nb * 8 * 65:(nb + 1) * 8 * 65] \
                                    .rearrange("p (h c) -> p h c", c=65)[:, :, 0:DK]
                                nc.any.tensor_copy(dst, src)
                            dst1 = vh[m][:].rearrange(
                                "p (h c) -> p h c", c=65)[:, :, DK:65]
                            nc.vector.tensor_copy(
                                dst1,
                                ones16[:].rearrange("p (h c) -> p h c", c=1))

            do_tensor(v_d, wv_d, "bv", "nat", vh)
            do_tensor(q_d, wq_d, "bq", "T", qhT)
            do_tensor(k_d, wk_d, "bk", "T", khT)

        # ===================== C: attention per head pair ====================
        with tc.tile_pool(name="c_sbuf", bufs=1) as cp, \
             tc.tile_pool(name="c_psum", bufs=1, space="PSUM") as cps:
            R4 = (0, 32, 64, 96)
            for hc in range(DC):
                hA, hB = 2 * hc, 2 * hc + 1
                qh2 = (qhT[hc][0:64, :], qhT[hc][64:128, :])
                kh2 = (khT[hc][0:64, :], khT[hc][64:128, :])

                # ---- C1: transposed pass + PV ----
                ctx_ps = [[cps.tile([65, NB], F32, tag=f"ctx{Xi}{ib}", bufs=1,
                                    name=f"ctx{Xi}{ib}")
                           for ib in range(2)] for Xi in range(2)]
                for jc in range(SC):
                    st2 = [cps.tile([P, S], F32, tag="st", bufs=2, name="st2")
                           for _ in range(2)]
                    for ib in range(2):
                        for Xi in range(2):
                            nc.tensor.matmul(
                                st2[Xi][:, ib * NB:(ib + 1) * NB],
                                kh2[Xi][:, jc * P:(jc + 1) * P],
                                qh2[Xi][:, ib * NB:(ib + 1) * NB],
                                start=True, stop=True)
                    pT2 = [cp.tile([P, S], F32R, tag="pT", bufs=3, name="pT2")
                           for _ in range(2)]
                    for Xi in range(2):
                        nc.scalar.activation(
                            pT2[Xi][:], st2[Xi][:], EXP,
                            bias=mask_colneg[:, jc:jc + 1], scale=SCALE)
                    for ib in range(2):
                        for Xi, h in ((0, hA), (1, hB)):
                            nc.tensor.matmul(
                                ctx_ps[Xi][ib][:],
                                vh[jc][:, h * 65:(h + 1) * 65],
                                pT2[Xi][:, ib * NB:(ib + 1) * NB],
                                start=(jc == 0), stop=(jc == SC - 1))

                # ---- C1 epilogue: L, 1/L, normalize ctx ----
                linvT2 = []
                for Xi, h in ((0, hA), (1, hB)):
                    l_row = cp.tile([1, S], F32, tag="lrow", bufs=1)
                    for ib in range(2):
                        nc.any.tensor_copy(
                            l_row[0:1, ib * NB:(ib + 1) * NB],
                            ctx_ps[Xi][ib][64:65, :])
                    linv = cp.tile([1, S], F32, tag="linv", bufs=1)
                    nc.vector.reciprocal(linv[:], l_row[:])
                    linv_r = cp.tile([1, S], F32R, tag="linvr", bufs=1)
                    nc.vector.tensor_copy(linv_r[:], linv[:])
                    linvT = cp.tile([P, SC], F32, tag=f"linvT{Xi}", bufs=2,
                                    name=f"linvT{Xi}")
                    for t in range(SC):
                        tl = cps.tile([P, 1], F32, tag="st", bufs=2)
                        nc.tensor.transpose(
                            tl[:], linv[0:1, t * P:(t + 1) * P],
                            ones_f[0:1, 0:1])
                        nc.any.tensor_copy(linvT[:, t:t + 1], tl[:])
                    linvT2.append(linvT)
                    r0 = (h % 2) * 64
                    for ib in range(2):
                        lbc_ps = cps.tile([64, NB], F32, tag="st", bufs=2)
                        nc.tensor.matmul(
                            lbc_ps[:], ones4[0:1, 0:64],
                            linv_r[0:1, ib * NB:(ib + 1) * NB],
                            start=True, stop=True)
                        lbc = cp.tile([64, NB], F32, tag="lbc", bufs=2)
                        nc.any.tensor_copy(lbc[:], lbc_ps[:])
                        nc.vector.tensor_mul(
                            ctxT[hc][r0:r0 + 64, ib * NB:(ib + 1) * NB],
                            ctx_ps[Xi][ib][0:64, :], lbc[:])

                # ---- C2: normal pass -> attn output ----
                for t in range(SC):
                    sp2 = [cps.tile([P, S], F32, tag="st", bufs=2, name="sp2")
                           for _ in range(2)]
                    for jb in range(2):
                        for Xi in range(2):
                            nc.tensor.matmul(
                                sp2[Xi][:, jb * NB:(jb + 1) * NB],
                                qh2[Xi][:, t * P:(t + 1) * P],
                                kh2[Xi][:, jb * NB:(jb + 1) * NB],
                                start=True, stop=False)
                        for Xi in range(2):
                            r = R4[2 * jb + Xi]
                            nc.tensor.matmul(
                                sp2[Xi][:, jb * NB:(jb + 1) * NB],
                                ones4[r:r + 1, 0:P],
                                mask4[r:r + 1, jb * NB:(jb + 1) * NB],
                                start=False, stop=True,
                                tile_position=_tile_pos(r))
                    for Xi, h in ((0, hA), (1, hB)):
                        pa = cp.tile([P, S], F32, tag="pa", bufs=3, name="pa")
                        nc.scalar.activation(pa[:], sp2[Xi][:], EXP,
                                             scale=SCALE)
                        nc.vector.tensor_scalar_mul(
                            pa[:], pa[:], linvT2[Xi][:, t:t + 1])
                        nc.sync.dma_start(
                            attn_d[h, t * P:(t + 1) * P, :], pa[:])

        # ===================== D: output projection ==========================
        with tc.tile_pool(name="d_sbuf", bufs=1) as dp, \
             tc.tile_pool(name="d_psum", bufs=1, space="PSUM") as dps:
            wo_sb = []
            for c in range(DC):
                wt = dp.tile([P, D], F32R, tag=f"wo{c}", bufs=1, name=f"wo{c}")
                nc.sync.dma_start(wt[:], wo_d.bitcast(F32R)[c * P:(c + 1) * P, :])
                wo_sb.append(wt)
            for t in range(SC):
                outst = dp.tile([P, D], F32, tag="outst", bufs=2)
                for nb in range(2):
                    op = dps.tile([P, NB], F32, tag="op", bufs=4)
                    for c in range(DC):
                        nc.tensor.matmul(
                            op[:], ctxT[c][:, t * P:(t + 1) * P],
                            wo_sb[c][:, nb * NB:(nb + 1) * NB],
                            start=(c == 0), stop=False)
                    nc.tensor.matmul(
                        op[:], ones4[96:97, 0:P],
                        b4r[96:97, nb * NB:(nb + 1) * NB],
                        start=False, stop=True, tile_position=(96, 0))
                    nc.any.tensor_copy(outst[:, nb * NB:(nb + 1) * NB], op[:])
                nc.sync.dma_start(out_d[t * P:(t + 1) * P, :], outst[:])

    nc.compile()
    return nc


def get_module():
    if "nc" not in _CACHE:
        _CACHE["nc"] = _build_module()
    return _CACHE["nc"]


def make_in_maps(q, k, v, mask, wq, bq, wk, bk, wv, bv, wo, bo):
    q = np.asarray(q, np.float32)
    k = np.asarray(k, np.float32)
    v = np.asarray(v, np.float32)
    mask = np.asarray(mask, np.float32).reshape(q.shape[0], -1)
    shared = {
        "wq": np.ascontiguousarray(wq, dtype=np.float32),
        "wk": np.ascontiguousarray(wk, dtype=np.float32),
        "wv": np.ascontiguousarray(wv, dtype=np.float32),
        "wo": np.ascontiguousarray(wo, dtype=np.float32),
        "bq": np.ascontiguousarray(bq, dtype=np.float32),
        "bk": np.ascontiguousarray(bk, dtype=np.float32),
        "bv": np.ascontiguousarray(bv, dtype=np.float32),
        "bo": np.ascontiguousarray(bo, dtype=np.float32),
        "ident": np.eye(P, dtype=np.float32),
    }
    return [
        {"q": np.ascontiguousarray(q[b]), "k": np.ascontiguousarray(k[b]),
         "v": np.ascontiguousarray(v[b]),
         "mask": np.ascontiguousarray(mask[b]), **shared}
        for b in range(N_CORES)
    ]


def kernel(q, k, v, mask, wq, bq, wk, bk, wv, bv, wo, bo):
    nc = get_module()
    in_maps = make_in_maps(q, k, v, mask, wq, bq, wk, bk, wv, bv, wo, bo)
    res = bass_utils.run_bass_kernel_spmd(nc, in_maps, list(range(N_CORES)))
    out = np.stack([res.results[b]["out"] for b in range(N_CORES)])
    attn = np.stack([res.results[b]["attn"] for b in range(N_CORES)])
    return out, attn


# revision 13
# speedup vs baseline: 1.2414x; 1.2414x over previous
"""Multi-head attention TRN2 Bass kernel, batch-parallel over 8 NeuronCores.

Problem (hardcoded): B=8, Sq=Sk=1024, D_MODEL=1024, H=16, DK=DV=64,
DIM_OUT=1024, f32.  mask (B,1,1,Sk) marks keys to suppress (x*-1e9).
Returns (out, attn) like the reference.

Per-core plan (core b handles batch element b; no collectives):
  A) load+PE-transpose v,q,k (streamed in seq halves)
  B) projections:
       qhT = (q@wq+bq)^T   [H*DK, Sq]   (transposed layout)
       khT = (k@wk+bk)^T   [H*DK, Sk]
       vh  = v@wv+bv       [Sk, H*(DV+1)] (natural; per-head 65th col = 1.0
                                           so PV also row-sums exp -> L)
  C) per head pair (2 heads share a 128-partition chunk; K=64 matmuls of the
     two heads pack onto PE row groups):
     C1 transposed pass: S^T[j,i] tiles -> exp(0.125*S^T + NEG*mask[j]) (mask
        as per-partition ACT bias) -> P^T -> PV: ctx^T[d,i] (+L row via the
        ones column).  ctx normalized by 1/L (k=1 ones-matmul broadcast)
        into ctxT (float32r) for the output projection.
     C2 normal pass: S[i,j] tiles (+ mask*NEG*8 added via k=1 matmul
        accumulate, pre-scale) -> exp(0.125*S') -> *1/L[i] (per-partition
        scalar) -> DMA to attn output.
  D) out = ctxN^T.T @ wo + bo -> DMA out.

All matmuls run in float32r (fp32 stored, PE rounds mantissa, 1 cycle/row);
PSUM accumulation is fp32.
"""

import numpy as np
from contextlib import ExitStack

import concourse.bass as bass
import concourse.tile as tile
from concourse import bacc, mybir
from concourse import bass_utils

F32 = mybir.dt.float32
F32R = mybir.dt.float32r
EXP = mybir.ActivationFunctionType.Exp

P = 128          # partitions
S = 1024         # seq len (q and k)
D = 1024         # d_model
H = 16           # heads
DK = 64          # head dim
NB = 512         # matmul moving-dim block (fp32 max)
SC = S // P      # 8 seq chunks
DC = D // P      # 8 d_model chunks
NEG = -1e9
SCALE = 0.125    # 1/sqrt(DK)
N_CORES = 8

# bias rows inside the consolidated [P, S] bias tiles
BROW = {"bq": 0, "bk": 32, "bv": 64, "bo": 96}

_CACHE = {}


def _tile_pos(r):
    return (r, 0) if r == 96 else None


def _build_module(with_biases=True):
    nc = bacc.Bacc("TRN2", target_bir_lowering=False, debug=False)

    q_d = nc.dram_tensor("q", [S, D], F32, kind="ExternalInput").ap()
    k_d = nc.dram_tensor("k", [S, D], F32, kind="ExternalInput").ap()
    v_d = nc.dram_tensor("v", [S, D], F32, kind="ExternalInput").ap()
    mask_d = nc.dram_tensor("mask", [S], F32, kind="ExternalInput").ap()
    wq_d = nc.dram_tensor("wq", [D, H * DK], F32, kind="ExternalInput").ap()
    wk_d = nc.dram_tensor("wk", [D, H * DK], F32, kind="ExternalInput").ap()
    wv_d = nc.dram_tensor("wv", [D, H * DK], F32, kind="ExternalInput").ap()
    wo_d = nc.dram_tensor("wo", [H * DK, D], F32, kind="ExternalInput").ap()
    b_ds = {
        "bq": nc.dram_tensor("bq", [H * DK], F32, kind="ExternalInput").ap(),
        "bk": nc.dram_tensor("bk", [H * DK], F32, kind="ExternalInput").ap(),
        "bv": nc.dram_tensor("bv", [H * DK], F32, kind="ExternalInput").ap(),
        "bo": nc.dram_tensor("bo", [D], F32, kind="ExternalInput").ap(),
    }
    ident_d = nc.dram_tensor("ident", [P, P], F32, kind="ExternalInput").ap()

    attn_d = nc.dram_tensor("attn", [H, S, S], F32, kind="ExternalOutput").ap()
    out_d = nc.dram_tensor("out", [S, D], F32, kind="ExternalOutput").ap()
    linv_scr_d = nc.dram_tensor("linv_scr", [H, S], F32).ap()

    with tile.TileContext(nc) as tc, ExitStack() as top:
        const = top.enter_context(tc.tile_pool(name="const", bufs=1))
        persist = top.enter_context(tc.tile_pool(name="persist", bufs=1))

        # ---- constants ----
        ident_r = const.tile([P, P], F32R, tag="ident_r")
        nc.sync.dma_start(ident_r[:], ident_d.bitcast(F32R)[:])

        ones_f = const.tile([1, NB], F32, tag="ones_f")
        nc.vector.memset(ones_f[:], 1.0)
        ones4 = const.tile([P, NB], F32R, tag="ones4")
        for r in (0, 32, 64, 96):
            nc.vector.tensor_copy(ones4[r:r + 1, :], ones_f[:])
        mask4 = const.tile([P, S], F32R, tag="mask4")
        nc.sync.dma_start(mask4[0:1, :],
                          mask_d.bitcast(F32R).rearrange("(a n) -> a n", a=1))
        nc.vector.tensor_scalar_mul(
            mask4[0:1, :], mask4[0:1, :].bitcast(F32), NEG * 8.0)
        for r in (32, 64, 96):
            nc.vector.tensor_copy(mask4[r:r + 1, :], mask4[0:1, :].bitcast(F32))

        # mask in column layout: [p, c] = mask[c*128+p] * NEG  (ACT bias, C1)
        mask_colneg = const.tile([P, SC], F32, tag="mask_colneg")
        nc.sync.dma_start(mask_colneg[:], mask_d.rearrange("(c p) -> p c", p=P))
        nc.vector.tensor_scalar_mul(mask_colneg[:], mask_colneg[:], NEG)

        # biases on partitions {0,32,64,96} of one [P, S] tile
        b4r = const.tile([P, S], F32R, tag="b4r")
        for nm, r in BROW.items():
            nc.sync.dma_start(
                b4r[r:r + 1, :],
                b_ds[nm].bitcast(F32R).rearrange("(a n) -> a n", a=1))

        # ---- persistent big tensors ----
        qhT = [persist.tile([P, S], F32R, tag=f"qhT{c}", name=f"qhT{c}")
               for c in range(DC)]
        khT = [persist.tile([P, S], F32R, tag=f"khT{c}", name=f"khT{c}")
               for c in range(DC)]
        vh = [persist.tile([P, S], F32R, tag=f"vh{c}", name=f"vh{c}")
              for c in range(SC)]
        ctxT = [persist.tile([P, S], F32R, tag=f"ctxT{c}", name=f"ctxT{c}")
                for c in range(DC)]

        # ================= A+B: transpose inputs and project =================
        with tc.tile_pool(name="ab_sbuf", bufs=1) as abp, \
             tc.tile_pool(name="ab_psum", bufs=1, space="PSUM") as abps:

            def do_tensor(x_d, w_d, bnm, mode, dstT):
                ws = []
                for c in range(DC):
                    wt = abp.tile([P, D], F32R, tag=f"w{c}", bufs=1,
                                  name=f"w{c}")
                    nc.sync.dma_start(
                        wt[:], w_d.bitcast(F32R)[c * P:(c + 1) * P, :])
                    ws.append(wt)
                br = BROW[bnm]
                for half in range(2):
                    # transpose seq rows [half*512, half*512+512) -> xT halves
                    xT = [abp.tile([P, NB], F32R, tag=f"xT{c}", bufs=1,
                                   name=f"xT{c}") for c in range(DC)]
                    for tp_i in range(2):
                        t0 = half * 4 + 2 * tp_i
                        stg0 = abp.tile([P, D], F32R, tag="stg0", bufs=2)
                        nc.sync.dma_start(
                            stg0[:], x_d.bitcast(F32R)[t0 * P:(t0 + 1) * P, :])
                        stg1 = abp.tile([P, D], F32R, tag="stg1", bufs=2)
                        nc.sync.dma_start(
                            stg1[:],
                            x_d.bitcast(F32R)[(t0 + 1) * P:(t0 + 2) * P, :])
                        for c in range(DC):
                            tp = abps.tile([P, 2 * P], F32R, tag="tp", bufs=4)
                            nc.tensor.transpose(
                                tp[:, 0:P], stg0[:, c * P:(c + 1) * P],
                                ident_r[:])
                            nc.tensor.transpose(
                                tp[:, P:2 * P], stg1[:, c * P:(c + 1) * P],
                                ident_r[:])
                            nc.vector.tensor_copy(
                                xT[c][:, tp_i * 2 * P:(tp_i + 1) * 2 * P],
                                tp[:])
                    if mode == "T":
                        # dstT[m][:, half*NB:...] = (x@w+b)^T block
                        nb = half
                        for m in range(DC):
                            ps = abps.tile([P, NB], F32, tag="proj", bufs=4)
                            for c in range(DC):
                                nc.tensor.matmul(
                                    ps[:], ws[c][:, m * P:(m + 1) * P],
                                    xT[c][:], start=(c == 0),
                                    stop=(not with_biases and c == DC - 1))
                            if with_biases:
                                nc.tensor.matmul(
                                    ps[:], b4r[br:br + 1, m * P:(m + 1) * P],
                                    ones4[br:br + 1, 0:NB],
                                    start=False, stop=True,
                                    tile_position=_tile_pos(br))
                            nc.any.tensor_copy(
                                dstT[m][:, nb * NB:(nb + 1) * NB], ps[:])
                    else:
                        # natural layout: vh[j-tile m in this half][cols]
                        for m in range(half * 4, half * 4 + 4):
                            for nb in range(2):
                                ps = abps.tile([P, NB], F32, tag="proj",
                                               bufs=4)
                                for c in range(DC):
                                    nc.tensor.matmul(
                                        ps[:],
                                        xT[c][:, (m % 4) * P:(m % 4 + 1) * P],
                                        ws[c][:, nb * NB:(nb + 1) * NB],
                                        start=(c == 0),
                                        stop=(not with_biases and c == DC - 1))
                                if with_biases:
                                    nc.tensor.matmul(
                                        ps[:], ones4[br:br + 1, 0:P],
                                        b4r[br:br + 1, nb * NB:(nb + 1) * NB],
                                        start=False, stop=True,
                                        tile_position=_tile_pos(br))
                                nc.any.tensor_copy(
                                    vh[m][:, nb * NB:(nb + 1) * NB], ps[:])

            do_tensor(v_d, wv_d, "bv", "nat", vh)
            do_tensor(q_d, wq_d, "bq", "T", qhT)
            do_tensor(k_d, wk_d, "bk", "T", khT)

        # ===================== C: attention per head pair ====================
        with tc.tile_pool(name="c_sbuf", bufs=1) as cp, \
             tc.tile_pool(name="c_psum", bufs=1, space="PSUM") as cps:
            R4 = (0, 32, 64, 96)
            for hc in range(DC):
                hA, hB = 2 * hc, 2 * hc + 1
                qh2 = (qhT[hc][0:64, :], qhT[hc][64:128, :])
                kh2 = (khT[hc][0:64, :], khT[hc][64:128, :])

                # ---- C2 first (independent of C1): normal pass ->
                #      attn output, with L rows via ACT accum_out.
                lcols2 = [cp.tile([P, SC], F32, tag=f"lcol{Xi}", bufs=2,
                                  name=f"lcol{Xi}") for Xi in range(2)]
                licols2 = [cp.tile([P, SC], F32, tag=f"licol{Xi}", bufs=2,
                                   name=f"licol{Xi}") for Xi in range(2)]
                for t in range(SC):
                    sp2 = [cps.tile([P, S], F32, tag="st", bufs=2, name="sp2")
                           for _ in range(2)]
                    for jb in range(2):
                        for Xi in range(2):
                            nc.tensor.matmul(
                                sp2[Xi][:, jb * NB:(jb + 1) * NB],
                                qh2[Xi][:, t * P:(t + 1) * P],
                                kh2[Xi][:, jb * NB:(jb + 1) * NB],
                                start=True, stop=False)
                        for Xi in range(2):
                            r = R4[2 * jb + Xi]
                            nc.tensor.matmul(
                                sp2[Xi][:, jb * NB:(jb + 1) * NB],
                                ones4[r:r + 1, 0:P],
                                mask4[r:r + 1, jb * NB:(jb + 1) * NB],
                                start=False, stop=True,
                                tile_position=_tile_pos(r))
                    for Xi, h in ((0, hA), (1, hB)):
                        pa = cp.tile([P, S], F32, tag="pa", bufs=4, name="pa")
                        nc.scalar.activation(
                            pa[:], sp2[Xi][:], EXP, scale=SCALE,
                            accum_out=lcols2[Xi][:, t:t + 1])
                        nc.vector.reciprocal(
                            licols2[Xi][:, t:t + 1], lcols2[Xi][:, t:t + 1])
                        nc.vector.tensor_scalar_mul(
                            pa[:], pa[:], licols2[Xi][:, t:t + 1])
                        nc.sync.dma_start(
                            attn_d[h, t * P:(t + 1) * P, :], pa[:])
                # 1/L rows to DRAM scratch (read back broadcast for ctx)
                for Xi, h in ((0, hA), (1, hB)):
                    nc.sync.dma_start(
                        linv_scr_d[h].rearrange("(c p) -> p c", p=P),
                        licols2[Xi][:])

                # ---- C1: transposed pass + PV ----
                ctx_ps = [[cps.tile([64, NB], F32, tag=f"ctx{Xi}{ib}", bufs=1,
                                    name=f"ctx{Xi}{ib}")
                           for ib in range(2)] for Xi in range(2)]
                for jc in range(SC):
                    st2 = [cps.tile([P, S], F32, tag="st", bufs=2, name="st2")
                           for _ in range(2)]
                    for ib in range(2):
                        for Xi in range(2):
                            nc.tensor.matmul(
                                st2[Xi][:, ib * NB:(ib + 1) * NB],
                                kh2[Xi][:, jc * P:(jc + 1) * P],
                                qh2[Xi][:, ib * NB:(ib + 1) * NB],
                                start=True, stop=True)
                    pT2 = [cp.tile([P, S], F32R, tag="pT", bufs=3, name="pT2")
                           for _ in range(2)]
                    for Xi in range(2):
                        nc.scalar.activation(
                            pT2[Xi][:], st2[Xi][:], EXP,
                            bias=mask_colneg[:, jc:jc + 1], scale=SCALE)
                    for ib in range(2):
                        for Xi, h in ((0, hA), (1, hB)):
                            nc.tensor.matmul(
                                ctx_ps[Xi][ib][:],
                                vh[jc][:, h * DK:(h + 1) * DK],
                                pT2[Xi][:, ib * NB:(ib + 1) * NB],
                                start=(jc == 0), stop=(jc == SC - 1))

                # ---- normalize ctx by 1/L (broadcast via DRAM bounce) ----
                for Xi, h in ((0, hA), (1, hB)):
                    for ib in range(2):
                        lbc = cp.tile([64, NB], F32, tag="lbc", bufs=2)
                        nc.sync.dma_start(
                            lbc[:],
                            linv_scr_d[h][ib * NB:(ib + 1) * NB].rearrange(
                                "(a n) -> a n", a=1).partition_broadcast(64))
                        nc.vector.tensor_mul(
                            ctxT[hc][(h % 2) * 64:(h % 2) * 64 + 64,
                                     ib * NB:(ib + 1) * NB],
                            ctx_ps[Xi][ib][:], lbc[:])

            # ============ D: output projection (inside C scope) ============
            # wo is loaded in column halves so it can coexist with the
            # C-phase pools; PSUM comes from the shared "st" tag.
            for nbh in range(2):
                wo_sb = []
                for c in range(DC):
                    wt = cp.tile([P, NB], F32R, tag=f"wo{c}", bufs=1,
                                 name=f"wo{c}")
                    nc.sync.dma_start(
                        wt[:], wo_d.bitcast(F32R)[c * P:(c + 1) * P,
                                                  nbh * NB:(nbh + 1) * NB])
                    wo_sb.append(wt)
                for t in range(SC):
                    op = cps.tile([P, NB], F32, tag="st", bufs=2, name="op")
                    for c in range(DC):
                        nc.tensor.matmul(
                            op[:], ctxT[c][:, t * P:(t + 1) * P],
                            wo_sb[c][:], start=(c == 0),
                            stop=(not with_biases and c == DC - 1))
                    if with_biases:
                        nc.tensor.matmul(
                            op[:], ones4[96:97, 0:P],
                            b4r[96:97, nbh * NB:(nbh + 1) * NB],
                            start=False, stop=True, tile_position=(96, 0))
                    outst = cp.tile([P, NB], F32, tag="outst", bufs=3,
                                    name="outst")
                    nc.any.tensor_copy(outst[:], op[:])
                    nc.sync.dma_start(
                        out_d[t * P:(t + 1) * P, nbh * NB:(nbh + 1) * NB],
                        outst[:])

    nc.compile()
    return nc


def get_module(with_biases=True):
    key = ("nc", with_biases)
    if key not in _CACHE:
        _CACHE[key] = _build_module(with_biases)
    return _CACHE[key]


def make_in_maps(q, k, v, mask, wq, bq, wk, bk, wv, bv, wo, bo):
    q = np.asarray(q, np.float32)
    k = np.asarray(k, np.float32)
    v = np.asarray(v, np.float32)
    mask = np.asarray(mask, np.float32).reshape(q.shape[0], -1)
    shared = {
        "wq": np.ascontiguousarray(wq, dtype=np.float32),
        "wk": np.ascontiguousarray(wk, dtype=np.float32),
        "wv": np.ascontiguousarray(wv, dtype=np.float32),
        "wo": np.ascontiguousarray(wo, dtype=np.float32),
        "bq": np.ascontiguousarray(bq, dtype=np.float32),
        "bk": np.ascontiguousarray(bk, dtype=np.float32),
        "bv": np.ascontiguousarray(bv, dtype=np.float32),
        "bo": np.ascontiguousarray(bo, dtype=np.float32),
        "ident": np.eye(P, dtype=np.float32),
    }
    return [
        {"q": np.ascontiguousarray(q[b]), "k": np.ascontiguousarray(k[b]),
         "v": np.ascontiguousarray(v[b]),
         "mask": np.ascontiguousarray(mask[b]), **shared}
        for b in range(N_CORES)
    ]


def kernel(q, k, v, mask, wq, bq, wk, bk, wv, bv, wo, bo):
    with_biases = any(
        np.any(np.asarray(b)) for b in (bq, bk, bv, bo))
    nc = get_module(with_biases)
    in_maps = make_in_maps(q, k, v, mask, wq, bq, wk, bk, wv, bv, wo, bo)
    res = bass_utils.run_bass_kernel_spmd(nc, in_maps, list(range(N_CORES)))
    out = np.stack([res.results[b]["out"] for b in range(N_CORES)])
    attn = np.stack([res.results[b]["attn"] for b in range(N_CORES)])
    return out, attn
